# revision 78
# baseline (speedup 1.0000x reference)
"""MultiHeadCrossAttention on 8 TRN2 NeuronCores.

Sharding: tensor-parallel over heads (16 heads -> 2 per core); host sums
the 8 output partials. All-bf16 datapath (fp32 PSUM accumulation), which
halves HBM traffic and keeps every matmul at 1.0 PE-cycles/row.

Per core, the 8 (batch, 512-q-group) stages run through a 4-deep software
pipeline, emitted in 8 "pieces" per slot with projections of later batches
interleaved:
  S(stage s+1):  S.T[kv,q] = K-slice @ Q.T-slice (2x512 cols per kc pair);
                 P = exp(S/8) -> bf16 [128kv, 1024] tiles. S runs one slot
                 ahead so the Act engine's exps always have a slot of slack.
  PV(stage s-1): full-PE-utilization flipped matmuls, trailing the slot
                 start by 2 pieces: out[q,65] += P-block.T @ [V|1]
                 (stationary = P [128kv,128q], moving = [V|ones] [128kv,65];
                 the ones column accumulates the softmax denominator).
                 Normalize via per-partition reciprocal, then DMA-XBAR
                 transpose [q,dd] -> [dd,q] (PE transpose for the two drain
                 stages, whose shorter latency chain suits the thin tail).
  O(stage s-3):  Y.T[E,q] partial = WoT-slice.T @ attnT -> bf16 staging ->
                 one DMA per stage (qb-granular + split stores in the drain).
V is projected directly in [kv, feat] layout (stationary = X2.T chunks,
moving = WvT) so no on-device V transposes are needed. NOTE: V-tile writes
use strided views whose deps are not reliably enforced against PV reads --
keep V-projection units >= 1 slot ahead of the PV stage that reads them
(violating this produced hardware races).

fp8/DoubleRow variants were measured and rejected: exp(S) needs more than
fp8 mantissa (rel err 2.7e-2+ vs the 2e-2 budget); bf16 everywhere lands
at 7.1e-3. Cost-model time 184.2us vs 254.0us for the fp32r baseline; PE
busy time sits at the bf16 matmul roofline (164.7us); the remainder is
DMA-bandwidth-bound startup (~7us, the first x slabs stream in column/ec
chunks ahead of their consumers) and pipeline-drain latency (~6us; drain
out-projections borrow the idle score-psum ring for double buffering, and
tail stores go out in oc-pair quarters gated on each copy).
"""
import numpy as np
from contextlib import ExitStack

import ml_dtypes

import concourse.bass as bass
import concourse.mybir as mybir
import concourse.tile as tile
from concourse import bacc
from concourse.bass_utils import run_bass_kernel_spmd

N_CORES = 8
B, SQ, SKV, E, DH = 4, 1024, 2048, 1024, 64
Q_ROWS = B * SQ      # 4096
KV_ROWS = B * SKV    # 8192
EC = E // 128        # 8 contraction chunks
QC = Q_ROWS // 512   # 8 q slabs
KVC_B = SKV // 128   # 16 kv chunks (128-wide) per batch
NSLAB = SKV // 512   # 4 kv slabs (512-wide) per batch
BF16 = mybir.dt.bfloat16
F32 = mybir.dt.float32
FP8 = mybir.dt.float8e4
Exp = mybir.ActivationFunctionType.Exp
DR = mybir.MatmulPerfMode.DoubleRow
NPBF = ml_dtypes.bfloat16

# fp8 P/V for a DoubleRow PV matmul was tried and fails the accuracy
# budget (softmax weights need >3 mantissa bits; measured 2.7e-2+ vs the
# 2e-2 tolerance), so the datapath stays bf16.
PV_FP8 = False

_CACHE = {}


def _build(n_reps=1):
    nc = bacc.Bacc("TRN2", target_bir_lowering=False, debug=False,
                   num_devices=N_CORES)
    x1t = nc.dram_tensor("x1t", [QC, 128, EC, 512], BF16,
                         kind="ExternalInput").ap()
    x2t = nc.dram_tensor("x2t", [B * NSLAB, 128, EC, 512], BF16,
                         kind="ExternalInput").ap()
    wqt = nc.dram_tensor("wqt", [128, EC, 128], BF16, kind="ExternalInput").ap()
    wkt = nc.dram_tensor("wkt", [128, EC, 128], BF16, kind="ExternalInput").ap()
    wvt = nc.dram_tensor("wvt", [128, EC, 128], BF16, kind="ExternalInput").ap()
    wot = nc.dram_tensor("wot", [128, E], BF16, kind="ExternalInput").ap()
    bqv = nc.dram_tensor("bq", [128, 1], F32, kind="ExternalInput").ap()
    bkv = nc.dram_tensor("bk", [128, 1], F32, kind="ExternalInput").ap()
    bvb = nc.dram_tensor("bvb", [128, 128], F32, kind="ExternalInput").ap()
    onv = nc.dram_tensor("ones", [128, 1], F32, kind="ExternalInput").ap()
    ngv = nc.dram_tensor("negone", [128, 1], F32, kind="ExternalInput").ap()
    idv = nc.dram_tensor("ident", [128, 128], BF16, kind="ExternalInput").ap()
    yt = nc.dram_tensor("yt", [E, Q_ROWS], BF16, kind="ExternalOutput").ap()
    yt_r = yt.rearrange("(oc p) q -> p oc q", p=128)

    with tile.TileContext(nc) as tc, ExitStack() as ctx:
        const = ctx.enter_context(tc.tile_pool(name="const", bufs=1))
        persist = ctx.enter_context(tc.tile_pool(name="persist", bufs=1))
        xload = ctx.enter_context(tc.tile_pool(name="xload", bufs=4))
        ptp = ctx.enter_context(tc.tile_pool(name="ptp", bufs=48))
        work = ctx.enter_context(tc.tile_pool(name="work", bufs=2))
        ps_s = ctx.enter_context(tc.tile_pool(name="ps_s", bufs=2, space="PSUM"))
        ps_o = ctx.enter_context(tc.tile_pool(name="ps_o", bufs=2, space="PSUM"))
        ps_pj = ctx.enter_context(tc.tile_pool(name="ps_pj", bufs=2,
                                               space="PSUM"))

        wq_sb = const.tile([128, EC, 128], BF16, tag="wq")
        wk_sb = const.tile([128, EC, 128], BF16, tag="wk")
        wv_sb = const.tile([128, EC, 128], BF16, tag="wv")
        wo_sb = const.tile([128, E], BF16, tag="wo")
        bq_sb = const.tile([128, 1], F32, tag="bq")
        bk_sb = const.tile([128, 1], F32, tag="bk")
        bvb_sb = const.tile([128, 128], F32, tag="bvb")
        ones_sb = const.tile([128, 1], F32, tag="ones1")
        neg1_sb = const.tile([128, 1], F32, tag="neg1")
        id_sb = const.tile([128, 128], BF16, tag="id")

        for rep in range(n_reps):
            VDT = FP8 if PV_FP8 else BF16
            qt_sb = persist.tile([128, Q_ROWS], BF16, tag="qt",
                                 name=f"qt_{rep}")
            kt_sb = [persist.tile([128, SKV], BF16, tag=f"kt{b}",
                                  name=f"kt{b}_{rep}") for b in range(B)]
            v_sb = [persist.tile([128, KVC_B, 130], VDT, tag=f"v{b}",
                                 name=f"vz{b}_{rep}") for b in range(B)]

            xstash = {}

            def unit_q(j, xt=None):
                if xt is None:
                    xt = xload.tile([128, EC, 512], BF16, tag="x",
                                    name=f"xq{j}_{rep}")
                    nc.sync.dma_start(xt[:], x1t[j])
                ps = ps_pj.tile([128, 512], F32, tag="pj",
                                name=f"qps{j}_{rep}")
                for ec in range(EC):
                    nc.tensor.matmul(ps[:], wq_sb[:, ec], xt[:, ec],
                                     start=(ec == 0), stop=(ec == EC - 1))
                nc.vector.tensor_scalar_add(qt_sb[:, j * 512:(j + 1) * 512],
                                            ps[:], bq_sb[:])

            def unit_k(b, jj, xt=None):
                if xt is None:
                    xt = xload.tile([128, EC, 512], BF16, tag="x",
                                    name=f"xkv{b}_{jj}_{rep}")
                    nc.sync.dma_start(xt[:], x2t[b * NSLAB + jj])
                xstash[(b, jj)] = xt
                ps = ps_pj.tile([128, 512], F32, tag="pj",
                                name=f"kps{b}_{jj}_{rep}")
                for ec in range(EC):
                    nc.tensor.matmul(ps[:], wk_sb[:, ec], xt[:, ec],
                                     start=(ec == 0), stop=(ec == EC - 1))
                nc.vector.tensor_scalar_add(
                    kt_sb[b][:, jj * 512:(jj + 1) * 512], ps[:], bk_sb[:])

            def unit_v(b, jj):
                xt = xstash.pop((b, jj))
                ps = ps_pj.tile([128, 512], F32, tag="pj",
                                name=f"vps{b}_{jj}_{rep}")
                for t in range(4):
                    st = xt[:, :, t * 128:(t + 1) * 128]
                    for ec in range(EC):
                        nc.tensor.matmul(ps[:, t * 128:(t + 1) * 128],
                                         st[:, ec], wv_sb[:, ec],
                                         start=(ec == 0), stop=(ec == EC - 1))
                psv = ps[:].rearrange("p (kc s y) -> p kc s y", kc=4, s=2)
                dst = v_sb[b][:, jj * 4:(jj + 1) * 4].rearrange(
                    "p kc (s y) -> p kc s y", s=2)[:, :, :, 0:64]
                bvv = bvb_sb[:].rearrange("p (s y) -> p s y", s=2) \
                    .unsqueeze(1).to_broadcast((128, 4, 2, 64))
                nc.vector.tensor_add(dst, psv, bvv)

            pt_tiles = {}
            atq_tiles = {}
            att_tiles = {}
            ysb_tiles = {}

            def s_piece(si, k):
                b, g = divmod(si, 2)
                qs = slice(si * 512, (si + 1) * 512)
                for h in range(2):
                    hp = h * 64
                    sp = ps_s.tile([128, 1024], F32, tag="s",
                                   name=f"sps{si}_{k}_{h}_{rep}")
                    for u in range(2):
                        kc = 2 * k + u
                        nc.tensor.matmul(
                            sp[:, u * 512:(u + 1) * 512],
                            kt_sb[b][hp:hp + 64, kc * 128:(kc + 1) * 128],
                            qt_sb[hp:hp + 64, qs],
                            start=True, stop=True)
                    pt = ptp.tile([128, 1024], FP8 if PV_FP8 else BF16,
                                  tag="pt", name=f"pt{si}_{k}_{h}_{rep}")
                    if PV_FP8:
                        nc.scalar.activation(pt[:], sp[:], Exp,
                                             bias=neg1_sb[:], scale=0.125)
                    else:
                        nc.scalar.activation(pt[:], sp[:], Exp, scale=0.125)
                    pt_tiles[(si, k, h)] = pt

            def pv_group(si, k):
                b, g = divmod(si, 2)
                qb, h = divmod(k, 2)
                op = ps_o.tile([128, 65], F32, tag="o",
                               name=f"ops{si}_{k}_{rep}")
                c0 = qb * 128
                for kp in range(8):
                    pt = pt_tiles[(si, kp, h)]
                    if PV_FP8:
                        # DoubleRow: both kv chunks of the pair in one matmul
                        nc.tensor.matmul(
                            op[:],
                            pt[:].rearrange("p (u q) -> p u q",
                                            u=2)[:, :, c0:c0 + 128],
                            v_sb[b][:, 2 * kp:2 * kp + 2,
                                    h * 65:h * 65 + 65],
                            start=(kp == 0), stop=(kp == 7),
                            perf_mode=DR)
                    else:
                        for u in range(2):
                            nc.tensor.matmul(
                                op[:],
                                pt[:, u * 512 + c0:u * 512 + c0 + 128],
                                v_sb[b][:, 2 * kp + u, h * 65:h * 65 + 65],
                                start=(kp == 0 and u == 0),
                                stop=(kp == 7 and u == 1))
                rc = work.tile([128, 1], F32, tag="rc", bufs=4,
                               name=f"rc{si}_{k}_{rep}")
                nc.vector.reciprocal(rc[:], op[:, 64:65])
                if h == 0:
                    atq_tiles[(si, qb)] = work.tile(
                        [128, 128], BF16, tag="atq", bufs=4,
                        name=f"atq{si}_{qb}_{rep}")
                nc.vector.tensor_scalar_mul(
                    atq_tiles[(si, qb)][:, h * 64:(h + 1) * 64],
                    op[:, 0:64], rc[:])

            def transpose_piece(si, qb):
                if qb == 0:
                    att_tiles[si] = work.tile([128, 512], BF16, tag="att",
                                              bufs=3, name=f"att{si}_{rep}")
                dst = att_tiles[si][:, qb * 128:(qb + 1) * 128]
                src = atq_tiles.pop((si, qb))
                if si >= 6:
                    # drain: PE transpose (short latency); steady state uses
                    # the DMA XBAR (latency hidden by pipeline depth)
                    tp = ps_pj.tile([128, 128], BF16, tag="pj", bufs=2,
                                    name=f"tp{si}_{qb}_{rep}")
                    nc.tensor.transpose(tp[:], src[:], id_sb[:])
                    nc.vector.tensor_copy(dst, tp[:])
                else:
                    nc.sync.dma_start(dst, src[:], transpose=True)

            def oproj_piece(si, oc):
                # in the drain the S-score psum ring is idle; alternating
                # into it doubles the effective buffering for O matmuls
                pool = ps_s if si >= 5 and oc % 2 == 1 else ps_pj
                tagn = "s" if si >= 5 and oc % 2 == 1 else "pj"
                yp = pool.tile([128, 512], F32, tag=tagn,
                               name=f"yps{si}_{oc}_{rep}")
                nc.tensor.matmul(yp[:], wo_sb[:, oc * 128:(oc + 1) * 128],
                                 att_tiles[si][:], start=True, stop=True)
                if oc == 0:
                    ysb_tiles[si] = work.tile([128, EC, 512], BF16, tag="ysb",
                                              bufs=3, name=f"ysb{si}_{rep}")
                if si >= 5:
                    # drain slots: Act is past its last exps and otherwise idle
                    nc.scalar.copy(ysb_tiles[si][:, oc], yp[:])
                else:
                    nc.vector.tensor_copy(ysb_tiles[si][:, oc], yp[:])
                if si >= 4 and oc % 2 == 1:
                    # late stages: store in oc-pair quarters as copies land so
                    # no big transfer sits in front of the final store
                    nc.sync.dma_start(
                        yt_r[:, oc - 1:oc + 1, si * 512:(si + 1) * 512],
                        ysb_tiles[si][:, oc - 1:oc + 1])
                    if oc == EC - 1:
                        ysb_tiles.pop(si)
                        del att_tiles[si]
                elif oc == EC - 1:
                    nc.sync.dma_start(
                        yt_r[:, :, si * 512:(si + 1) * 512],
                        ysb_tiles.pop(si)[:])
                    del att_tiles[si]

            def oproj_qb(si, qb):
                # drain stages: qb-granular so oproj chases the transposes
                if qb == 0:
                    ysb_tiles[si] = work.tile([128, EC, 512], BF16, tag="ysb",
                                              bufs=3, name=f"ysb{si}_{rep}")
                cq = slice(qb * 128, (qb + 1) * 128)
                for half in range(2):
                    pool = ps_s if half == 1 else ps_pj
                    tagn = "s" if half == 1 else "pj"
                    yp = pool.tile([128, 512], F32, tag=tagn,
                                   name=f"yqps{si}_{qb}_{half}_{rep}")
                    for j in range(4):
                        oc = half * 4 + j
                        nc.tensor.matmul(
                            yp[:, j * 128:(j + 1) * 128],
                            wo_sb[:, oc * 128:(oc + 1) * 128],
                            att_tiles[si][:, cq], start=True, stop=True)
                    ydst = ysb_tiles[si][:, half * 4:(half + 1) * 4, cq]
                    ysrc = yp[:].rearrange("p (j q) -> p j q", j=4)
                    if si == 7 and not (qb == 3 and half == 0) or \
                            si == 6 and qb <= 2:
                        nc.scalar.copy(ydst, ysrc)
                    else:
                        nc.vector.tensor_copy(ydst, ysrc)
                    if si == 7 and qb == 3:
                        # tail: store each oc-half as soon as its copy lands
                        nc.sync.dma_start(
                            yt_r[:, half * 4:(half + 1) * 4,
                                 si * 512 + 256:si * 512 + 512],
                            ysb_tiles[si][:, half * 4:(half + 1) * 4,
                                          256:512])
                c0 = si * 512
                if si == 7 and qb == 3:
                    ysb_tiles.pop(si)
                    del att_tiles[si]
                elif qb == 1:
                    nc.sync.dma_start(yt_r[:, :, c0:c0 + 256],
                                      ysb_tiles[si][:, :, 0:256])
                elif qb == 3:
                    nc.sync.dma_start(yt_r[:, :, c0 + 256:c0 + 512],
                                      ysb_tiles.pop(si)[:, :, 256:512])
                    del att_tiles[si]

            # per-slot projection unit lists (deadline-safe schedule)
            pro_units = [("k", 0, 1), ("v", 0, 1), ("k", 0, 2),
                         ("v", 0, 2), ("k", 0, 3), ("v", 0, 3)]
            slot_units = [
                [("q", 2, 0), ("k", 1, 0), ("v", 1, 0), ("k", 1, 1),
                 ("v", 1, 1)],
                [("q", 3, 0), ("k", 1, 2), ("v", 1, 2), ("k", 1, 3),
                 ("v", 1, 3)],
                [("q", 4, 0), ("k", 2, 0), ("v", 2, 0), ("k", 2, 1),
                 ("v", 2, 1)],
                [("q", 5, 0), ("k", 2, 2), ("v", 2, 2), ("k", 2, 3),
                 ("v", 2, 3)],
                [("q", 6, 0), ("k", 3, 0), ("v", 3, 0), ("k", 3, 1),
                 ("v", 3, 1)],
                [("q", 7, 0), ("k", 3, 2), ("v", 3, 2), ("k", 3, 3),
                 ("v", 3, 3)],
                [], [], [], [],
            ]

            def run_unit(u):
                kind, a, bb = u
                if kind == "q":
                    unit_q(a)
                elif kind == "k":
                    unit_k(a, bb)
                else:
                    unit_v(a, bb)

            # prologue: DMA order minimizes time-to-first-matmul; x slabs
            # stream in 2-ec chunks paced against the consuming matmuls
            xt_q0 = xload.tile([128, EC, 512], BF16, tag="x",
                               name=f"xq0_{rep}")
            xt_k0 = xload.tile([128, EC, 512], BF16, tag="x",
                               name=f"xkv0_0_{rep}")
            nc.sync.dma_start(wq_sb[:, 0:2], wqt[:, 0:2])
            nc.sync.dma_start(xt_q0[:, :, 0:256], x1t[0][:, :, 0:256])
            nc.sync.dma_start(wq_sb[:, 2:8], wqt[:, 2:8])
            nc.sync.dma_start(bq_sb[:], bqv[:])
            nc.sync.dma_start(wk_sb[:], wkt[:])
            nc.sync.dma_start(xt_k0[:, :, 0:256], x2t[0][:, :, 0:256])
            nc.sync.dma_start(bk_sb[:], bkv[:])
            nc.sync.dma_start(xt_q0[:, :, 256:512], x1t[0][:, :, 256:512])
            nc.sync.dma_start(xt_k0[:, :, 256:512], x2t[0][:, :, 256:512])
            nc.sync.dma_start(wv_sb[:], wvt[:])
            nc.sync.dma_start(bvb_sb[:], bvb[:])
            nc.sync.dma_start(ones_sb[:], onv[:])
            # Q/K projections of the first slabs run in column halves fed by
            # column-sliced DMAs, and S(0,0) runs in matching q-halves, so
            # the first score matmuls (and the Act engine's exp stream) start
            # ~3us earlier than a whole-slab schedule allows.
            xstash[(0, 0)] = xt_k0
            sp0 = {}
            pt0 = {}

            def qk0_half(half):
                cs = slice(half * 256, (half + 1) * 256)
                psq = ps_pj.tile([128, 256], F32, tag="pj",
                                 name=f"qps0h{half}_{rep}")
                for ec in range(EC):
                    nc.tensor.matmul(psq[:], wq_sb[:, ec], xt_q0[:, ec, cs],
                                     start=(ec == 0), stop=(ec == EC - 1))
                nc.vector.tensor_scalar_add(qt_sb[:, cs], psq[:], bq_sb[:])
                psk = ps_pj.tile([128, 256], F32, tag="pj",
                                 name=f"kps00h{half}_{rep}")
                for ec in range(EC):
                    nc.tensor.matmul(psk[:], wk_sb[:, ec], xt_k0[:, ec, cs],
                                     start=(ec == 0), stop=(ec == EC - 1))
                nc.vector.tensor_scalar_add(kt_sb[0][:, cs], psk[:], bk_sb[:])

            def s00_half(half):
                qs = slice(half * 256, half * 256 + 256)
                for h in range(2):
                    hp = h * 64
                    if half == 0:
                        sp0[h] = ps_s.tile([128, 1024], F32, tag="s",
                                           name=f"sps0_0h_{h}_{rep}")
                        pt0[h] = ptp.tile([128, 1024], BF16, tag="pt",
                                          name=f"pt0_0_{h}_{rep}")
                        pt_tiles[(0, 0, h)] = pt0[h]
                    for u in range(2):
                        nc.tensor.matmul(
                            sp0[h][:, u * 512 + half * 256:
                                   u * 512 + half * 256 + 256],
                            kt_sb[0][hp:hp + 64, u * 128:(u + 1) * 128],
                            qt_sb[hp:hp + 64, qs], start=True, stop=True)
                    sv = sp0[h][:].rearrange("p (u q) -> p u q", u=2)[
                        :, :, half * 256:half * 256 + 256]
                    pv = pt0[h][:].rearrange("p (u q) -> p u q", u=2)[
                        :, :, half * 256:half * 256 + 256]
                    nc.scalar.activation(pv, sv, Exp, scale=0.125)
            nc.sync.dma_start(wo_sb[:], wot[:])
            nc.sync.dma_start(id_sb[:], idv[:])
            nc.sync.dma_start(neg1_sb[:], ngv[:])
            # ones columns (softmax denominator trick): col 64 of each
            # 65-col [V_h | 1] block
            for b in range(B):
                vv = v_sb[b][:].rearrange("p kc (s y) -> p (kc s) y", y=65)
                nc.vector.tensor_copy(vv[:, :, 64:65],
                                      ones_sb[:].unsqueeze(-1)
                                      .to_broadcast((128, 2 * KVC_B, 1)))
            # S runs one slot ahead of the PV/O pipeline so the exps (Act
            # engine) always have a full slot of slack. S(0,0..1) depend only
            # on kt0 slab 0, so they are hoisted ahead of q1's DMA-paced
            # matmuls to start the Act engine ~4us earlier.
            qk0_half(0)
            s00_half(0)
            qk0_half(1)
            s00_half(1)
            unit_v(0, 0)
            s_piece(0, 1)
            run_unit(pro_units[0])
            run_unit(pro_units[1])
            unit_q(1)
            for kp in range(2, 8):
                if kp < len(pro_units):
                    run_unit(pro_units[kp])
                s_piece(0, kp)

            for s in range(10):
                units = list(slot_units[s])
                for k in range(8):
                    if k < len(units):
                        run_unit(units[k])
                    if s < 7:
                        s_piece(s + 1, k)
                    if 1 <= s <= 7:
                        # PV trails S by 2 pieces so the slot-boundary exps
                        # (Act engine) stay ahead of their PV consumers
                        if k >= 2:
                            pv_group(s - 1, k - 2)
                            if k % 2 == 1:
                                transpose_piece(s - 1, (k - 3) // 2)
                        if k == 7:
                            pv_group(s - 1, 6)
                            pv_group(s - 1, 7)
                            transpose_piece(s - 1, 3)
                    elif s == 8:
                        pv_group(s - 1, k)
                        if k % 2 == 1:
                            transpose_piece(s - 1, k // 2)
                    if 3 <= s <= 8:
                        oproj_piece(s - 3, k)
                    if s == 7 and k in (3, 5, 7):
                        oproj_qb(6, (k - 3) // 2)
                    elif s == 8 and k == 1:
                        oproj_qb(6, 3)
                    elif s == 8 and k in (3, 5, 7):
                        oproj_qb(7, (k - 3) // 2)
                    elif s == 9 and k == 0:
                        oproj_qb(7, 3)

    nc.compile()
    return nc


def _get_nc(n_reps=1):
    key = n_reps
    if key not in _CACHE:
        _CACHE[key] = _build(n_reps)
    return _CACHE[key]


def _tile_x(xt2d, nchunks):
    # [E, R] -> [R/512, 128, EC, 512] bf16:
    # x[j, p, ec, q] = xt2d[ec*128+p, j*512+q]
    return np.ascontiguousarray(
        xt2d.reshape(EC, 128, nchunks, 512).transpose(2, 1, 0, 3)).astype(NPBF)


def _tile_w(wt_slice):
    # [E, 128] -> [128, EC, 128]
    return np.ascontiguousarray(
        wt_slice.reshape(EC, 128, 128).transpose(1, 0, 2)).astype(NPBF)


def make_in_maps(x1, x2, Wq, bq, Wk, bk, Wv, bv, Wo, bo=None):
    x1 = np.asarray(x1, dtype=np.float32)
    x2 = np.asarray(x2, dtype=np.float32)
    x1t = _tile_x(np.ascontiguousarray(x1.reshape(Q_ROWS, E).T), QC)
    x2t = _tile_x(np.ascontiguousarray(x2.reshape(KV_ROWS, E).T),
                  KV_ROWS // 512)
    WqT = np.asarray(Wq, dtype=np.float32).T
    WkT = np.asarray(Wk, dtype=np.float32).T
    WvT = np.asarray(Wv, dtype=np.float32).T
    WoT = np.ascontiguousarray(np.asarray(Wo, dtype=np.float32).T)
    ones = np.ones((128, 1), dtype=np.float32)
    bqf = np.asarray(bq, np.float32)
    bkf = np.asarray(bk, np.float32)
    bvf = np.asarray(bv, np.float32)
    in_maps = []
    for c in range(N_CORES):
        s = slice(128 * c, 128 * (c + 1))
        in_maps.append({
            "x1t": x1t, "x2t": x2t,
            "wqt": _tile_w(WqT[:, s]),
            "wkt": _tile_w(WkT[:, s]),
            "wvt": _tile_w(WvT[:, s]),
            "wot": np.ascontiguousarray(WoT[s, :]).astype(NPBF),
            "bq": np.ascontiguousarray(bqf[s]).reshape(128, 1),
            "bk": np.ascontiguousarray(bkf[s]).reshape(128, 1),
            "bvb": np.ascontiguousarray(
                np.broadcast_to(bvf[s][None, :], (128, 128))),
            "ones": ones,
            "negone": -ones,
            "ident": np.eye(128, dtype=NPBF),
        })
    return in_maps


def kernel(x1, x2, Wq, bq, Wk, bk, Wv, bv, Wo, bo):
    nc = _get_nc()
    in_maps = make_in_maps(x1, x2, Wq, bq, Wk, bk, Wv, bv, Wo)
    res = run_bass_kernel_spmd(nc, in_maps, list(range(N_CORES)))
    ytf = res.results[0]["yt"].astype(np.float64)
    for c in range(1, N_CORES):
        ytf += res.results[c]["yt"].astype(np.float64)
    y = ytf.T.astype(np.float32) + np.asarray(bo, np.float32)[None, :]
    return y.reshape(B, SQ, E)


# revision 79
# speedup vs baseline: 1.0014x; 1.0014x over previous
"""MultiHeadCrossAttention on 8 TRN2 NeuronCores.

Sharding: tensor-parallel over heads (16 heads -> 2 per core); host sums
the 8 output partials. All-bf16 datapath (fp32 PSUM accumulation), which
halves HBM traffic and keeps every matmul at 1.0 PE-cycles/row.

Per core, the 8 (batch, 512-q-group) stages run through a 4-deep software
pipeline, emitted in 8 "pieces" per slot with projections of later batches
interleaved:
  S(stage s+1):  S.T[kv,q] = K-slice @ Q.T-slice (2x512 cols per kc pair);
                 P = exp(S/8) -> bf16 [128kv, 1024] tiles. S runs one slot
                 ahead so the Act engine's exps always have a slot of slack.
  PV(stage s-1): full-PE-utilization flipped matmuls, trailing the slot
                 start by 2 pieces: out[q,65] += P-block.T @ [V|1]
                 (stationary = P [128kv,128q], moving = [V|ones] [128kv,65];
                 the ones column accumulates the softmax denominator).
                 Normalize via per-partition reciprocal, then DMA-XBAR
                 transpose [q,dd] -> [dd,q] (PE transpose for the two drain
                 stages, whose shorter latency chain suits the thin tail).
  O(stage s-3):  Y.T[E,q] partial = WoT-slice.T @ attnT -> bf16 staging ->
                 one DMA per stage (qb-granular + split stores in the drain).
V is projected directly in [kv, feat] layout (stationary = X2.T chunks,
moving = WvT) so no on-device V transposes are needed. NOTE: V-tile writes
use strided views whose deps are not reliably enforced against PV reads --
keep V-projection units >= 1 slot ahead of the PV stage that reads them
(violating this produced hardware races).

fp8/DoubleRow variants were measured and rejected: exp(S) needs more than
fp8 mantissa (rel err 2.7e-2+ vs the 2e-2 budget); bf16 everywhere lands
at 7.1e-3. Cost-model time 184.2us vs 254.0us for the fp32r baseline; PE
busy time sits at the bf16 matmul roofline (164.7us); the remainder is
DMA-bandwidth-bound startup (~7us, the first x slabs stream in column/ec
chunks ahead of their consumers) and pipeline-drain latency (~6us; drain
out-projections borrow the idle score-psum ring for double buffering, and
tail stores go out in oc-pair quarters gated on each copy).
"""
import numpy as np
from contextlib import ExitStack

import ml_dtypes

import concourse.bass as bass
import concourse.mybir as mybir
import concourse.tile as tile
from concourse import bacc
from concourse.bass_utils import run_bass_kernel_spmd

N_CORES = 8
B, SQ, SKV, E, DH = 4, 1024, 2048, 1024, 64
Q_ROWS = B * SQ      # 4096
KV_ROWS = B * SKV    # 8192
EC = E // 128        # 8 contraction chunks
QC = Q_ROWS // 512   # 8 q slabs
KVC_B = SKV // 128   # 16 kv chunks (128-wide) per batch
NSLAB = SKV // 512   # 4 kv slabs (512-wide) per batch
BF16 = mybir.dt.bfloat16
F32 = mybir.dt.float32
FP8 = mybir.dt.float8e4
Exp = mybir.ActivationFunctionType.Exp
DR = mybir.MatmulPerfMode.DoubleRow
NPBF = ml_dtypes.bfloat16

# fp8 P/V for a DoubleRow PV matmul was tried and fails the accuracy
# budget (softmax weights need >3 mantissa bits; measured 2.7e-2+ vs the
# 2e-2 tolerance), so the datapath stays bf16.
PV_FP8 = False

_CACHE = {}


def _build(n_reps=1):
    nc = bacc.Bacc("TRN2", target_bir_lowering=False, debug=False,
                   num_devices=N_CORES)
    x1t = nc.dram_tensor("x1t", [QC, 128, EC, 512], BF16,
                         kind="ExternalInput").ap()
    x2t = nc.dram_tensor("x2t", [B * NSLAB, 128, EC, 512], BF16,
                         kind="ExternalInput").ap()
    wqt = nc.dram_tensor("wqt", [128, EC, 128], BF16, kind="ExternalInput").ap()
    wkt = nc.dram_tensor("wkt", [128, EC, 128], BF16, kind="ExternalInput").ap()
    wvt = nc.dram_tensor("wvt", [128, EC, 128], BF16, kind="ExternalInput").ap()
    wot = nc.dram_tensor("wot", [128, E], BF16, kind="ExternalInput").ap()
    bqv = nc.dram_tensor("bq", [128, 1], F32, kind="ExternalInput").ap()
    bkv = nc.dram_tensor("bk", [128, 1], F32, kind="ExternalInput").ap()
    bvb = nc.dram_tensor("bvb", [128, 128], F32, kind="ExternalInput").ap()
    onv = nc.dram_tensor("ones", [128, 1], F32, kind="ExternalInput").ap()
    ngv = nc.dram_tensor("negone", [128, 1], F32, kind="ExternalInput").ap()
    idv = nc.dram_tensor("ident", [128, 128], BF16, kind="ExternalInput").ap()
    yt = nc.dram_tensor("yt", [E, Q_ROWS], BF16, kind="ExternalOutput").ap()
    yt_r = yt.rearrange("(oc p) q -> p oc q", p=128)

    with tile.TileContext(nc) as tc, ExitStack() as ctx:
        const = ctx.enter_context(tc.tile_pool(name="const", bufs=1))
        persist = ctx.enter_context(tc.tile_pool(name="persist", bufs=1))
        xload = ctx.enter_context(tc.tile_pool(name="xload", bufs=4))
        ptp = ctx.enter_context(tc.tile_pool(name="ptp", bufs=48))
        work = ctx.enter_context(tc.tile_pool(name="work", bufs=2))
        ps_s = ctx.enter_context(tc.tile_pool(name="ps_s", bufs=2, space="PSUM"))
        ps_o = ctx.enter_context(tc.tile_pool(name="ps_o", bufs=2, space="PSUM"))
        ps_pj = ctx.enter_context(tc.tile_pool(name="ps_pj", bufs=2,
                                               space="PSUM"))

        wq_sb = const.tile([128, EC, 128], BF16, tag="wq")
        wk_sb = const.tile([128, EC, 128], BF16, tag="wk")
        wv_sb = const.tile([128, EC, 128], BF16, tag="wv")
        wo_sb = const.tile([128, E], BF16, tag="wo")
        bq_sb = const.tile([128, 1], F32, tag="bq")
        bk_sb = const.tile([128, 1], F32, tag="bk")
        bvb_sb = const.tile([128, 128], F32, tag="bvb")
        ones_sb = const.tile([128, 1], F32, tag="ones1")
        neg1_sb = const.tile([128, 1], F32, tag="neg1")
        id_sb = const.tile([128, 128], BF16, tag="id")

        for rep in range(n_reps):
            VDT = FP8 if PV_FP8 else BF16
            qt_sb = persist.tile([128, Q_ROWS], BF16, tag="qt",
                                 name=f"qt_{rep}")
            kt_sb = [persist.tile([128, SKV], BF16, tag=f"kt{b}",
                                  name=f"kt{b}_{rep}") for b in range(B)]
            v_sb = [persist.tile([128, KVC_B, 130], VDT, tag=f"v{b}",
                                 name=f"vz{b}_{rep}") for b in range(B)]

            xstash = {}

            def unit_q(j, xt=None):
                if xt is None:
                    xt = xload.tile([128, EC, 512], BF16, tag="x",
                                    name=f"xq{j}_{rep}")
                    nc.sync.dma_start(xt[:], x1t[j])
                ps = ps_pj.tile([128, 512], F32, tag="pj",
                                name=f"qps{j}_{rep}")
                for ec in range(EC):
                    nc.tensor.matmul(ps[:], wq_sb[:, ec], xt[:, ec],
                                     start=(ec == 0), stop=(ec == EC - 1))
                nc.vector.tensor_scalar_add(qt_sb[:, j * 512:(j + 1) * 512],
                                            ps[:], bq_sb[:])

            def unit_k(b, jj, xt=None):
                if xt is None:
                    xt = xload.tile([128, EC, 512], BF16, tag="x",
                                    name=f"xkv{b}_{jj}_{rep}")
                    nc.sync.dma_start(xt[:], x2t[b * NSLAB + jj])
                xstash[(b, jj)] = xt
                ps = ps_pj.tile([128, 512], F32, tag="pj",
                                name=f"kps{b}_{jj}_{rep}")
                for ec in range(EC):
                    nc.tensor.matmul(ps[:], wk_sb[:, ec], xt[:, ec],
                                     start=(ec == 0), stop=(ec == EC - 1))
                nc.vector.tensor_scalar_add(
                    kt_sb[b][:, jj * 512:(jj + 1) * 512], ps[:], bk_sb[:])

            def unit_v(b, jj):
                xt = xstash.pop((b, jj))
                ps = ps_pj.tile([128, 512], F32, tag="pj",
                                name=f"vps{b}_{jj}_{rep}")
                for t in range(4):
                    st = xt[:, :, t * 128:(t + 1) * 128]
                    for ec in range(EC):
                        nc.tensor.matmul(ps[:, t * 128:(t + 1) * 128],
                                         st[:, ec], wv_sb[:, ec],
                                         start=(ec == 0), stop=(ec == EC - 1))
                psv = ps[:].rearrange("p (kc s y) -> p kc s y", kc=4, s=2)
                dst = v_sb[b][:, jj * 4:(jj + 1) * 4].rearrange(
                    "p kc (s y) -> p kc s y", s=2)[:, :, :, 0:64]
                bvv = bvb_sb[:].rearrange("p (s y) -> p s y", s=2) \
                    .unsqueeze(1).to_broadcast((128, 4, 2, 64))
                nc.vector.tensor_add(dst, psv, bvv)

            pt_tiles = {}
            atq_tiles = {}
            att_tiles = {}
            ysb_tiles = {}

            def s_piece(si, k):
                b, g = divmod(si, 2)
                qs = slice(si * 512, (si + 1) * 512)
                for h in range(2):
                    hp = h * 64
                    sp = ps_s.tile([128, 1024], F32, tag="s",
                                   name=f"sps{si}_{k}_{h}_{rep}")
                    for u in range(2):
                        kc = 2 * k + u
                        nc.tensor.matmul(
                            sp[:, u * 512:(u + 1) * 512],
                            kt_sb[b][hp:hp + 64, kc * 128:(kc + 1) * 128],
                            qt_sb[hp:hp + 64, qs],
                            start=True, stop=True)
                    pt = ptp.tile([128, 1024], FP8 if PV_FP8 else BF16,
                                  tag="pt", name=f"pt{si}_{k}_{h}_{rep}")
                    if PV_FP8:
                        nc.scalar.activation(pt[:], sp[:], Exp,
                                             bias=neg1_sb[:], scale=0.125)
                    else:
                        nc.scalar.activation(pt[:], sp[:], Exp, scale=0.125)
                    pt_tiles[(si, k, h)] = pt

            def pv_group(si, k):
                b, g = divmod(si, 2)
                qb, h = divmod(k, 2)
                op = ps_o.tile([128, 65], F32, tag="o",
                               name=f"ops{si}_{k}_{rep}")
                c0 = qb * 128
                for kp in range(8):
                    pt = pt_tiles[(si, kp, h)]
                    if PV_FP8:
                        # DoubleRow: both kv chunks of the pair in one matmul
                        nc.tensor.matmul(
                            op[:],
                            pt[:].rearrange("p (u q) -> p u q",
                                            u=2)[:, :, c0:c0 + 128],
                            v_sb[b][:, 2 * kp:2 * kp + 2,
                                    h * 65:h * 65 + 65],
                            start=(kp == 0), stop=(kp == 7),
                            perf_mode=DR)
                    else:
                        for u in range(2):
                            nc.tensor.matmul(
                                op[:],
                                pt[:, u * 512 + c0:u * 512 + c0 + 128],
                                v_sb[b][:, 2 * kp + u, h * 65:h * 65 + 65],
                                start=(kp == 0 and u == 0),
                                stop=(kp == 7 and u == 1))
                rc = work.tile([128, 1], F32, tag="rc", bufs=4,
                               name=f"rc{si}_{k}_{rep}")
                nc.vector.reciprocal(rc[:], op[:, 64:65])
                if h == 0:
                    atq_tiles[(si, qb)] = work.tile(
                        [128, 128], BF16, tag="atq", bufs=4,
                        name=f"atq{si}_{qb}_{rep}")
                nc.vector.tensor_scalar_mul(
                    atq_tiles[(si, qb)][:, h * 64:(h + 1) * 64],
                    op[:, 0:64], rc[:])

            def transpose_piece(si, qb):
                if qb == 0:
                    att_tiles[si] = work.tile([128, 512], BF16, tag="att",
                                              bufs=3, name=f"att{si}_{rep}")
                dst = att_tiles[si][:, qb * 128:(qb + 1) * 128]
                src = atq_tiles.pop((si, qb))
                if si >= 6:
                    # drain: PE transpose (short latency); steady state uses
                    # the DMA XBAR (latency hidden by pipeline depth)
                    tp = ps_pj.tile([128, 128], BF16, tag="pj", bufs=2,
                                    name=f"tp{si}_{qb}_{rep}")
                    nc.tensor.transpose(tp[:], src[:], id_sb[:])
                    nc.vector.tensor_copy(dst, tp[:])
                else:
                    nc.sync.dma_start(dst, src[:], transpose=True)

            def oproj_piece(si, oc):
                # in the drain the S-score psum ring is idle; alternating
                # into it doubles the effective buffering for O matmuls
                pool = ps_s if si >= 5 and oc % 2 == 1 else ps_pj
                tagn = "s" if si >= 5 and oc % 2 == 1 else "pj"
                yp = pool.tile([128, 512], F32, tag=tagn,
                               name=f"yps{si}_{oc}_{rep}")
                nc.tensor.matmul(yp[:], wo_sb[:, oc * 128:(oc + 1) * 128],
                                 att_tiles[si][:], start=True, stop=True)
                if oc == 0:
                    ysb_tiles[si] = work.tile([128, EC, 512], BF16, tag="ysb",
                                              bufs=3, name=f"ysb{si}_{rep}")
                if si >= 5:
                    # drain slots: Act is past its last exps and otherwise idle
                    nc.scalar.copy(ysb_tiles[si][:, oc], yp[:])
                else:
                    nc.vector.tensor_copy(ysb_tiles[si][:, oc], yp[:])
                if si >= 4 and oc % 2 == 1:
                    # late stages: store in oc-pair quarters as copies land so
                    # no big transfer sits in front of the final store
                    nc.sync.dma_start(
                        yt_r[:, oc - 1:oc + 1, si * 512:(si + 1) * 512],
                        ysb_tiles[si][:, oc - 1:oc + 1])
                    if oc == EC - 1:
                        ysb_tiles.pop(si)
                        del att_tiles[si]
                elif oc == EC - 1:
                    nc.sync.dma_start(
                        yt_r[:, :, si * 512:(si + 1) * 512],
                        ysb_tiles.pop(si)[:])
                    del att_tiles[si]

            def oproj_qb(si, qb):
                # drain stages: qb-granular so oproj chases the transposes
                if qb == 0:
                    ysb_tiles[si] = work.tile([128, EC, 512], BF16, tag="ysb",
                                              bufs=3, name=f"ysb{si}_{rep}")
                cq = slice(qb * 128, (qb + 1) * 128)
                for half in range(2):
                    pool = ps_s if half == 1 else ps_pj
                    tagn = "s" if half == 1 else "pj"
                    yp = pool.tile([128, 512], F32, tag=tagn,
                                   name=f"yqps{si}_{qb}_{half}_{rep}")
                    for j in range(4):
                        oc = half * 4 + j
                        nc.tensor.matmul(
                            yp[:, j * 128:(j + 1) * 128],
                            wo_sb[:, oc * 128:(oc + 1) * 128],
                            att_tiles[si][:, cq], start=True, stop=True)
                    ydst = ysb_tiles[si][:, half * 4:(half + 1) * 4, cq]
                    ysrc = yp[:].rearrange("p (j q) -> p j q", j=4)
                    if si == 7 and not (qb == 3 and half == 0) or \
                            si == 6 and qb <= 2:
                        nc.scalar.copy(ydst, ysrc)
                    else:
                        nc.vector.tensor_copy(ydst, ysrc)
                    if si == 7 and qb == 3:
                        # tail: store each oc-half as soon as its copy lands
                        nc.sync.dma_start(
                            yt_r[:, half * 4:(half + 1) * 4,
                                 si * 512 + 256:si * 512 + 512],
                            ysb_tiles[si][:, half * 4:(half + 1) * 4,
                                          256:512])
                c0 = si * 512
                if si == 7 and qb == 3:
                    ysb_tiles.pop(si)
                    del att_tiles[si]
                elif qb == 1:
                    nc.sync.dma_start(yt_r[:, :, c0:c0 + 256],
                                      ysb_tiles[si][:, :, 0:256])
                elif qb == 3:
                    nc.sync.dma_start(yt_r[:, :, c0 + 256:c0 + 512],
                                      ysb_tiles.pop(si)[:, :, 256:512])
                    del att_tiles[si]

            # per-slot projection unit lists (deadline-safe schedule)
            pro_units = [("k", 0, 1), ("v", 0, 1), ("k", 0, 2),
                         ("v", 0, 2), ("k", 0, 3), ("v", 0, 3)]
            slot_units = [
                [("q", 2, 0), ("k", 1, 0), ("v", 1, 0), ("k", 1, 1),
                 ("v", 1, 1)],
                [("q", 3, 0), ("k", 1, 2), ("v", 1, 2), ("k", 1, 3),
                 ("v", 1, 3)],
                [("q", 4, 0), ("k", 2, 0), ("v", 2, 0), ("k", 2, 1),
                 ("v", 2, 1)],
                [("q", 5, 0), ("k", 2, 2), ("v", 2, 2), ("k", 2, 3),
                 ("v", 2, 3)],
                [("q", 6, 0), ("k", 3, 0), ("v", 3, 0), ("k", 3, 1),
                 ("v", 3, 1)],
                [("q", 7, 0), ("k", 3, 2), ("v", 3, 2), ("k", 3, 3),
                 ("v", 3, 3)],
                [], [], [], [],
            ]

            def run_unit(u):
                kind, a, bb = u
                if kind == "q":
                    unit_q(a)
                elif kind == "k":
                    unit_k(a, bb)
                else:
                    unit_v(a, bb)

            # prologue: DMA order minimizes time-to-first-matmul; x slabs
            # stream in 2-ec chunks paced against the consuming matmuls
            xt_q0 = xload.tile([128, EC, 512], BF16, tag="x",
                               name=f"xq0_{rep}")
            xt_k0 = xload.tile([128, EC, 512], BF16, tag="x",
                               name=f"xkv0_0_{rep}")
            nc.sync.dma_start(wq_sb[:], wqt[:])
            nc.sync.dma_start(xt_q0[:, :, 0:256], x1t[0][:, :, 0:256])
            nc.sync.dma_start(bq_sb[:], bqv[:])
            nc.sync.dma_start(wk_sb[:], wkt[:])
            nc.sync.dma_start(xt_k0[:, :, 0:256], x2t[0][:, :, 0:256])
            nc.sync.dma_start(bk_sb[:], bkv[:])
            nc.sync.dma_start(xt_q0[:, :, 256:512], x1t[0][:, :, 256:512])
            nc.sync.dma_start(xt_k0[:, :, 256:512], x2t[0][:, :, 256:512])
            nc.sync.dma_start(wv_sb[:], wvt[:])
            nc.sync.dma_start(bvb_sb[:], bvb[:])
            nc.sync.dma_start(ones_sb[:], onv[:])
            # Q/K projections of the first slabs run in column halves fed by
            # column-sliced DMAs, and S(0,0) runs in matching q-halves, so
            # the first score matmuls (and the Act engine's exp stream) start
            # ~3us earlier than a whole-slab schedule allows.
            xstash[(0, 0)] = xt_k0
            sp0 = {}
            pt0 = {}

            def qk0_half(half):
                cs = slice(half * 256, (half + 1) * 256)
                psq = ps_pj.tile([128, 256], F32, tag="pj",
                                 name=f"qps0h{half}_{rep}")
                for ec in range(EC):
                    nc.tensor.matmul(psq[:], wq_sb[:, ec], xt_q0[:, ec, cs],
                                     start=(ec == 0), stop=(ec == EC - 1))
                nc.vector.tensor_scalar_add(qt_sb[:, cs], psq[:], bq_sb[:])
                psk = ps_pj.tile([128, 256], F32, tag="pj",
                                 name=f"kps00h{half}_{rep}")
                for ec in range(EC):
                    nc.tensor.matmul(psk[:], wk_sb[:, ec], xt_k0[:, ec, cs],
                                     start=(ec == 0), stop=(ec == EC - 1))
                nc.vector.tensor_scalar_add(kt_sb[0][:, cs], psk[:], bk_sb[:])

            def s00_half(half):
                qs = slice(half * 256, half * 256 + 256)
                for h in range(2):
                    hp = h * 64
                    if half == 0:
                        sp0[h] = ps_s.tile([128, 1024], F32, tag="s",
                                           name=f"sps0_0h_{h}_{rep}")
                        pt0[h] = ptp.tile([128, 1024], BF16, tag="pt",
                                          name=f"pt0_0_{h}_{rep}")
                        pt_tiles[(0, 0, h)] = pt0[h]
                    for u in range(2):
                        nc.tensor.matmul(
                            sp0[h][:, u * 512 + half * 256:
                                   u * 512 + half * 256 + 256],
                            kt_sb[0][hp:hp + 64, u * 128:(u + 1) * 128],
                            qt_sb[hp:hp + 64, qs], start=True, stop=True)
                    sv = sp0[h][:].rearrange("p (u q) -> p u q", u=2)[
                        :, :, half * 256:half * 256 + 256]
                    pv = pt0[h][:].rearrange("p (u q) -> p u q", u=2)[
                        :, :, half * 256:half * 256 + 256]
                    nc.scalar.activation(pv, sv, Exp, scale=0.125)
            nc.sync.dma_start(wo_sb[:], wot[:])
            nc.sync.dma_start(id_sb[:], idv[:])
            nc.sync.dma_start(neg1_sb[:], ngv[:])
            # ones columns (softmax denominator trick): col 64 of each
            # 65-col [V_h | 1] block
            for b in range(B):
                vv = v_sb[b][:].rearrange("p kc (s y) -> p (kc s) y", y=65)
                nc.vector.tensor_copy(vv[:, :, 64:65],
                                      ones_sb[:].unsqueeze(-1)
                                      .to_broadcast((128, 2 * KVC_B, 1)))
            # S runs one slot ahead of the PV/O pipeline so the exps (Act
            # engine) always have a full slot of slack. S(0,0..1) depend only
            # on kt0 slab 0, so they are hoisted ahead of q1's DMA-paced
            # matmuls to start the Act engine ~4us earlier.
            qk0_half(0)
            s00_half(0)
            qk0_half(1)
            s00_half(1)
            unit_v(0, 0)
            s_piece(0, 1)
            run_unit(pro_units[0])
            run_unit(pro_units[1])
            unit_q(1)
            for kp in range(2, 8):
                if kp < len(pro_units):
                    run_unit(pro_units[kp])
                s_piece(0, kp)

            for s in range(10):
                units = list(slot_units[s])
                for k in range(8):
                    if k < len(units):
                        run_unit(units[k])
                    if s < 7:
                        s_piece(s + 1, k)
                    if 1 <= s <= 7:
                        # PV trails S by 2 pieces so the slot-boundary exps
                        # (Act engine) stay ahead of their PV consumers
                        if k >= 2:
                            pv_group(s - 1, k - 2)
                            if k % 2 == 1:
                                transpose_piece(s - 1, (k - 3) // 2)
                        if k == 7:
                            pv_group(s - 1, 6)
                            pv_group(s - 1, 7)
                            transpose_piece(s - 1, 3)
                    elif s == 8:
                        pv_group(s - 1, k)
                        if k % 2 == 1:
                            transpose_piece(s - 1, k // 2)
                    if 3 <= s <= 8:
                        oproj_piece(s - 3, k)
                    if s == 7 and k in (3, 5, 7):
                        oproj_qb(6, (k - 3) // 2)
                    elif s == 8 and k == 1:
                        oproj_qb(6, 3)
                    elif s == 8 and k in (3, 5, 7):
                        oproj_qb(7, (k - 3) // 2)
                    elif s == 9 and k == 0:
                        oproj_qb(7, 3)

    nc.compile()
    return nc


def _get_nc(n_reps=1):
    key = n_reps
    if key not in _CACHE:
        _CACHE[key] = _build(n_reps)
    return _CACHE[key]


def _tile_x(xt2d, nchunks):
    # [E, R] -> [R/512, 128, EC, 512] bf16:
    # x[j, p, ec, q] = xt2d[ec*128+p, j*512+q]
    return np.ascontiguousarray(
        xt2d.reshape(EC, 128, nchunks, 512).transpose(2, 1, 0, 3)).astype(NPBF)


def _tile_w(wt_slice):
    # [E, 128] -> [128, EC, 128]
    return np.ascontiguousarray(
        wt_slice.reshape(EC, 128, 128).transpose(1, 0, 2)).astype(NPBF)


def make_in_maps(x1, x2, Wq, bq, Wk, bk, Wv, bv, Wo, bo=None):
    x1 = np.asarray(x1, dtype=np.float32)
    x2 = np.asarray(x2, dtype=np.float32)
    x1t = _tile_x(np.ascontiguousarray(x1.reshape(Q_ROWS, E).T), QC)
    x2t = _tile_x(np.ascontiguousarray(x2.reshape(KV_ROWS, E).T),
                  KV_ROWS // 512)
    WqT = np.asarray(Wq, dtype=np.float32).T
    WkT = np.asarray(Wk, dtype=np.float32).T
    WvT = np.asarray(Wv, dtype=np.float32).T
    WoT = np.ascontiguousarray(np.asarray(Wo, dtype=np.float32).T)
    ones = np.ones((128, 1), dtype=np.float32)
    bqf = np.asarray(bq, np.float32)
    bkf = np.asarray(bk, np.float32)
    bvf = np.asarray(bv, np.float32)
    in_maps = []
    for c in range(N_CORES):
        s = slice(128 * c, 128 * (c + 1))
        in_maps.append({
            "x1t": x1t, "x2t": x2t,
            "wqt": _tile_w(WqT[:, s]),
            "wkt": _tile_w(WkT[:, s]),
            "wvt": _tile_w(WvT[:, s]),
            "wot": np.ascontiguousarray(WoT[s, :]).astype(NPBF),
            "bq": np.ascontiguousarray(bqf[s]).reshape(128, 1),
            "bk": np.ascontiguousarray(bkf[s]).reshape(128, 1),
            "bvb": np.ascontiguousarray(
                np.broadcast_to(bvf[s][None, :], (128, 128))),
            "ones": ones,
            "negone": -ones,
            "ident": np.eye(128, dtype=NPBF),
        })
    return in_maps


def kernel(x1, x2, Wq, bq, Wk, bk, Wv, bv, Wo, bo):
    nc = _get_nc()
    in_maps = make_in_maps(x1, x2, Wq, bq, Wk, bk, Wv, bv, Wo)
    res = run_bass_kernel_spmd(nc, in_maps, list(range(N_CORES)))
    ytf = res.results[0]["yt"].astype(np.float64)
    for c in range(1, N_CORES):
        ytf += res.results[c]["yt"].astype(np.float64)
    y = ytf.T.astype(np.float32) + np.asarray(bo, np.float32)[None, :]
    return y.reshape(B, SQ, E)
